# revision 4
# baseline (speedup 1.0000x reference)
"""CronRootAttention (causal sqrt-N sparse attention + GQA projections) on 8 TRN2 cores.

Sharding: pure sequence shard — each core owns 256 queries, computes all 16
heads for them. Weights are replicated; k/v projections computed per-core for
the local 384-key span plus the 44 shared strided keys.

v2 dataflow (all matmuls bf16, f32 PSUM accumulate):
  - 6 input DMAs + 1 output DMA (vs 61): one 3D-AP DMA per tensor.
  - PE warmup stream during the initial DMA wait (p-state ramp).
  - projections: k_T [64,4g,428] (local+strided fused), q_T [64,16h,256],
    v rows in four overlapping 128-key blocks aligned to 64-query i-tiles.
  - attention per kv-group g: per i-tile one scores matmul covering all 4
    heads (moving = q[4h x 64q]); strided scores per head-pair.
  - softmax denominator per head-pair via an all-ones stationary matmul over
    the same p operands as PV: yields the denominator replicated across all
    128 partitions in one accumulation group; DVE reciprocal -> normalize is
    fused into the PSUM->SBUF evacuation multiply.
  - y rows [256,1024] = attn_T.T @ Wo chunks; single output DMA.
"""

import math
import sys

sys.path.insert(0, "/opt/trn_rl_repo")

import numpy as np
import concourse.bass as bass
import concourse.tile as tile
from concourse import bacc, mybir
from concourse.bass_utils import run_bass_kernel_spmd

F32 = mybir.dt.float32
BF16 = mybir.dt.bfloat16
EXP = mybir.ActivationFunctionType.Exp
import ml_dtypes

MM_DT = BF16
NP_DT = ml_dtypes.bfloat16

# Problem constants (hardcoded per contract).
B, S, D = 1, 2048, 1024
H, H_KV, HD = 16, 4, 64
W = int(math.ceil(math.sqrt(S)))  # 46
NCORES = 8
SQ = S // NCORES  # 256 queries per core
SKV = 384  # local key span: [qs-128, qs+256)
SIDX = np.arange(W - 1, S, W)  # strided key positions
NS = len(SIDX)  # 44
SA = SKV + NS  # 428: local + strided key columns, fused
KT = D // 128  # 8 contraction k-tiles
NWARM = 22  # PE warmup matmuls (p-state ramp during initial DMA wait)


def build_nc():
    nc = bacc.Bacc("TRN2", target_bir_lowering=False, debug=False, num_devices=1)
    xa = nc.dram_tensor("xa", [D, SA], MM_DT, kind="ExternalInput").ap()
    wq = nc.dram_tensor("wq", [D, D], MM_DT, kind="ExternalInput").ap()
    wk = nc.dram_tensor("wk", [D, 256], MM_DT, kind="ExternalInput").ap()
    wv = nc.dram_tensor("wv", [D, 256], MM_DT, kind="ExternalInput").ap()
    wo = nc.dram_tensor("wo", [D, D], MM_DT, kind="ExternalInput").ap()
    mall = nc.dram_tensor("mall", [128, 2048], MM_DT, kind="ExternalInput").ap()
    y = nc.dram_tensor("y", [SQ, D], F32, kind="ExternalOutput").ap()

    xa_r = xa.rearrange("(kt p) s -> p kt s", p=128)
    wq_r = wq.rearrange("(kt p) o -> p kt o", p=128)
    wk_r = wk.rearrange("(kt p) o -> p kt o", p=128)
    wv_r = wv.rearrange("(kt p) o -> p kt o", p=128)
    wo_r = wo.rearrange("(kt p) o -> p kt o", p=128)

    with tile.TileContext(nc) as tc:
        with (
            tc.tile_pool(name="consts", bufs=1) as consts,
            tc.tile_pool(name="work", bufs=1) as work,
        ):
            # ---- resident SBUF tensors, one DMA each ----
            xa_sb = consts.tile([128, KT, SA], MM_DT)
            wk_sb = consts.tile([128, KT, 256], MM_DT)
            wv_sb = consts.tile([128, KT, 256], MM_DT)
            wq_sb = consts.tile([128, KT, D], MM_DT)
            wo_sb = consts.tile([128, KT, D], MM_DT)
            mall_sb = consts.tile([128, 2048], MM_DT)
            nc.sync.dma_start(out=xa_sb[:], in_=xa_r)
            nc.sync.dma_start(out=wk_sb[:], in_=wk_r)
            nc.sync.dma_start(out=wv_sb[:], in_=wv_r)
            nc.sync.dma_start(out=wq_sb[:], in_=wq_r)
            nc.sync.dma_start(out=mall_sb[:], in_=mall)
            nc.sync.dma_start(out=wo_sb[:], in_=wo_r)
            mloc = mall_sb[:, 0:1024]  # [128j, (it,h,ii)]
            mstr = mall_sb[0:NS, 1024:2048]  # [44j, (h,i)]

            ones_sb = consts.tile([128, 128], MM_DT)
            nc.gpsimd.memset(ones_sb[:], 1.0)
            warm_a = consts.tile([1, 256], MM_DT)
            nc.gpsimd.memset(warm_a[:], 0.0)

            q_sb = work.tile([64, H, SQ], MM_DT)  # q_T per head (d on partitions)
            k_sb = work.tile([64, H_KV, SA], MM_DT)  # k_T per kv head, local+strided
            v_sb = work.tile([128, 4, 256], MM_DT)  # v rows, 4 overlapping key blocks
            vs_sb = work.tile([NS, 256], MM_DT)  # strided v rows
            attn_sb = work.tile([128, 8, SQ], MM_DT)  # normalized attn_T (2h per 128p)
            ys_sb = work.tile([128, 2, D], F32)

            # ---- phase A: PE warmup + projections ----
            with tc.tile_pool(name="ps_proj", bufs=4, space="PSUM") as psp:
                wp = psp.tile([128, 256], F32, tag="warm")
                for _ in range(NWARM):
                    nc.tensor.matmul(
                        wp[:], ones_sb[0:1, :], warm_a[:], start=True, stop=True
                    )
                for ot in range(2):  # k_T: 256 kv channels over 428 keys
                    kp = psp.tile([128, SA], F32, tag="proj")
                    for kt in range(KT):
                        nc.tensor.matmul(
                            kp[:],
                            wk_sb[:, kt, bass.ts(ot, 128)],
                            xa_sb[:, kt, :],
                            start=kt == 0,
                            stop=kt == KT - 1,
                        )
                    nc.scalar.copy(k_sb[:, 2 * ot, :], kp[0:64, :])
                    nc.scalar.copy(k_sb[:, 2 * ot + 1, :], kp[64:128, :])
                for bt in range(4):  # v rows: key block [64+64bt, 192+64bt)
                    vp = psp.tile([128, 256], F32, tag="proj")
                    for kt in range(KT):
                        nc.tensor.matmul(
                            vp[:],
                            xa_sb[:, kt, 64 + 64 * bt : 192 + 64 * bt],
                            wv_sb[:, kt, :],
                            start=kt == 0,
                            stop=kt == KT - 1,
                        )
                    nc.scalar.copy(v_sb[:, bt, :], vp[:])
                vsp = psp.tile([NS, 256], F32, tag="proj")
                for kt in range(KT):
                    nc.tensor.matmul(
                        vsp[:],
                        xa_sb[:, kt, SKV:SA],
                        wv_sb[:, kt, :],
                        start=kt == 0,
                        stop=kt == KT - 1,
                    )
                nc.scalar.copy(vs_sb[:], vsp[:])
                for ot in range(8):  # q_T
                    qp = psp.tile([128, SQ], F32, tag="proj")
                    for kt in range(KT):
                        nc.tensor.matmul(
                            qp[:],
                            wq_sb[:, kt, bass.ts(ot, 128)],
                            xa_sb[:, kt, 128:SKV],
                            start=kt == 0,
                            stop=kt == KT - 1,
                        )
                    nc.scalar.copy(q_sb[:, 2 * ot, :], qp[0:64, :])
                    nc.scalar.copy(q_sb[:, 2 * ot + 1, :], qp[64:128, :])

            # ---- phase B: sparse attention per kv-head group g ----
            with (
                tc.tile_pool(name="ps_sl", bufs=1, space="PSUM") as pssl,
                tc.tile_pool(name="ps_ss", bufs=1, space="PSUM") as psss,
                tc.tile_pool(name="ps_pv", bufs=2, space="PSUM") as pspv,
                tc.tile_pool(name="ps_den", bufs=2, space="PSUM") as psden,
                tc.tile_pool(name="ptiles", bufs=2) as pt,
                tc.tile_pool(name="rtiles", bufs=2) as rt,
            ):
                for g in range(4):
                    # local scores: per i-tile one matmul, 4 heads packed
                    sloc = pssl.tile([128, 4, 4, 64], F32, tag="sloc")
                    for it in range(4):
                        nc.tensor.matmul(
                            sloc[:, it, :, :],
                            k_sb[:, g, 64 + 64 * it : 192 + 64 * it],
                            q_sb[:, 4 * g : 4 * g + 4, 64 * it : 64 * it + 64],
                            start=True,
                            stop=True,
                        )
                    ploc = pt.tile([128, 4, 4, 64], MM_DT, tag="ploc")
                    sl2 = sloc.rearrange("p a b c -> p (a b c)")
                    pl2 = ploc.rearrange("p a b c -> p (a b c)")
                    nc.scalar.activation(pl2[:, 0:512], sl2[:, 0:512], EXP, scale=0.125)
                    nc.scalar.activation(
                        pl2[:, 512:1024], sl2[:, 512:1024], EXP, scale=0.125
                    )
                    nc.vector.tensor_mul(pl2[:], pl2[:], mloc)
                    # strided scores: per head-pair
                    sstr = psss.tile([NS, 4, 256], F32, tag="sstr")
                    for hp in range(2):
                        nc.tensor.matmul(
                            sstr[:, 2 * hp : 2 * hp + 2, :],
                            k_sb[:, g, SKV:SA],
                            q_sb[:, 4 * g + 2 * hp : 4 * g + 2 * hp + 2, :],
                            start=True,
                            stop=True,
                        )
                    pstr = pt.tile([NS, 4, 256], MM_DT, tag="pstr")
                    ss2 = sstr.rearrange("p a b -> p (a b)")
                    ps2 = pstr.rearrange("p a b -> p (a b)")
                    nc.scalar.activation(ps2[:, 0:512], ss2[:, 0:512], EXP, scale=0.125)
                    nc.scalar.activation(
                        ps2[:, 512:1024], ss2[:, 512:1024], EXP, scale=0.125
                    )
                    nc.vector.tensor_mul(ps2[:], ps2[:], mstr)

                    for hp in range(2):
                        h0 = 4 * g + 2 * hp
                        # denominator, replicated to all 128 partitions by the
                        # all-ones stationary
                        den = psden.tile([128, 2, 4, 64], F32, tag="den")
                        nc.tensor.matmul(
                            den.rearrange("p a b c -> p (a b c)"),
                            ones_sb[0:NS, :],
                            pstr[:, 2 * hp : 2 * hp + 2, :],
                            start=True,
                            stop=False,
                            skip_group_check=True,
                        )
                        for it in range(4):
                            nc.tensor.matmul(
                                den[:, :, it, :],
                                ones_sb[:],
                                ploc[:, it, 2 * hp : 2 * hp + 2, :],
                                start=False,
                                stop=it == 3,
                                skip_group_check=True,
                            )
                        # PV over the same p operands
                        pv = pspv.tile([64, 2, 4, 64], F32, tag="pv")
                        nc.tensor.matmul(
                            pv.rearrange("p a b c -> p (a b c)"),
                            vs_sb[:, 64 * g : 64 * g + 64],
                            pstr[:, 2 * hp : 2 * hp + 2, :],
                            start=True,
                            stop=False,
                            skip_group_check=True,
                        )
                        for it in range(4):
                            nc.tensor.matmul(
                                pv[:, :, it, :],
                                v_sb[:, it, 64 * g : 64 * g + 64],
                                ploc[:, it, 2 * hp : 2 * hp + 2, :],
                                start=False,
                                stop=it == 3,
                                skip_group_check=True,
                            )
                        rec = rt.tile([128, 2, SQ], F32, tag="rec")
                        with nc.allow_low_precision(reason="softmax denom recip"):
                            nc.vector.reciprocal(
                                rec.rearrange("p a b -> p (a b)"),
                                den.rearrange("p a b c -> p (a b c)"),
                            )
                        # normalize fused into PSUM->SBUF evacuation
                        pvf = pv.rearrange("p a b c -> p a (b c)")
                        nc.vector.tensor_mul(
                            attn_sb[0:64, 2 * g + hp, :], pvf[:, 0, :], rec[0:64, 0, :]
                        )
                        nc.vector.tensor_mul(
                            attn_sb[64:128, 2 * g + hp, :], pvf[:, 1, :], rec[0:64, 1, :]
                        )

            # ---- phase C: output projection ----
            with tc.tile_pool(name="ps_y", bufs=2, space="PSUM") as psy:
                for st in range(2):
                    for ch in range(2):
                        yp = psy.tile([128, 512], F32, tag="y")
                        for kt in range(KT):
                            nc.tensor.matmul(
                                yp[:],
                                attn_sb[:, kt, bass.ts(st, 128)],
                                wo_sb[:, kt, bass.ts(ch, 512)],
                                start=kt == 0,
                                stop=kt == KT - 1,
                            )
                        nc.scalar.copy(ys_sb[:, st, bass.ts(ch, 512)], yp[:])
                nc.sync.dma_start(
                    out=y.rearrange("(t p) o -> p t o", p=128), in_=ys_sb[:]
                )
    nc.compile()
    return nc


def host_prep(x, Wq, Wk, Wv, Wo):
    """Build per-core input maps (pure data reordering, no FLOPs)."""
    x2 = np.asarray(x, np.float32).reshape(S, D)
    xT = np.ascontiguousarray(x2.T)  # [D, S]
    xpad = np.zeros((D, 128 + S), np.float32)
    xpad[:, 128:] = xT
    xs = xT[:, SIDX]  # [D, 44]
    wq = np.ascontiguousarray(np.asarray(Wq, np.float32).T)
    wk = np.ascontiguousarray(np.asarray(Wk, np.float32).T)
    wv = np.ascontiguousarray(np.asarray(Wv, np.float32).T)
    wo = np.ascontiguousarray(np.asarray(Wo, np.float32).T)

    in_maps = []
    r = np.arange(128)
    ii = np.arange(64)
    iq = np.arange(SQ)
    for c in range(NCORES):
        qs = SQ * c
        xa = np.concatenate([xpad[:, qs : qs + SKV], xs], axis=1)  # [D, 428]
        mall = np.zeros((128, 2048), np.float32)
        for it in range(4):
            # query i = 64*it + ii (local), key row r -> local key 64+64*it+r,
            # global key qs + 64*it + r - 64
            diff = ii[None, :] - r[:, None] + 64
            jglob = qs + 64 * it + r[:, None] - 64
            msk = (diff >= 0) & (diff <= W - 1) & (jglob >= 0)
            for h in range(4):
                mall[:, 256 * it + 64 * h : 256 * it + 64 * h + 64] = msk
        mstr = (SIDX[:, None] <= qs + iq[None, :] - W).astype(np.float32)  # [44,256]
        mall[0:NS, 1024:2048] = np.tile(mstr, (1, 4))
        in_maps.append(
            {
                "xa": xa.astype(NP_DT),
                "wq": wq.astype(NP_DT),
                "wk": wk.astype(NP_DT),
                "wv": wv.astype(NP_DT),
                "wo": wo.astype(NP_DT),
                "mall": mall.astype(NP_DT),
            }
        )
    return in_maps


_NC_CACHE = {}


def get_nc():
    if "nc" not in _NC_CACHE:
        _NC_CACHE["nc"] = build_nc()
    return _NC_CACHE["nc"]


def kernel(x, Wq, Wk, Wv, Wo):
    nc = get_nc()
    in_maps = host_prep(x, Wq, Wk, Wv, Wo)
    res = run_bass_kernel_spmd(nc, in_maps, core_ids=list(range(NCORES)))
    yrows = np.concatenate([r["y"] for r in res.results], axis=0)  # [S, D]
    return np.ascontiguousarray(yrows).reshape(B, S, D).astype(np.float32)
